# revision 1
# baseline (speedup 1.0000x reference)
"""Multi-head attention kernel for Trainium2 (8 NeuronCores, SPMD).

Sharding: core c handles batch b=c//2 and 4 of the 8 heads
(projection columns 128*(c%2) .. +128).  Each core computes a partial
output projection (contracting only its own 128 head-dims); the host sums
the two partials per batch and adds bo.

Device algorithm (per core), S=2048, D=256, 4 heads of dh=32.  All hot
matmul operands are fp16 (1 cyc/row on the PE, fast weight load, PSUM
accumulation in fp32):
  qT/kT = (x @ W).T computed directly in [proj, S] layout
  v     = x @ Wv in natural [S, proj] layout, + ones column (den fused)
  scores^T[k,q] = sum_d kT[d,k] qT[d,q]   (4 heads row-tiled on the PE)
  mask bias (fp16 0 / -60000; exp underflows to exactly 0) applied either
    - on the PE: fp16 identity-matmul injected into the scores PSUM, or
    - on the DVE: tensor_add psum + mb -> sbuf        (split tunable)
  w = exp(scores) on ScalarE, fp16 out
  avT[d,q] + den[q] = [v | ones].T @ w    (col-tiled pairs, fp32 PSUM)
  o = av * (1/den)  (reciprocal_approx_fast + SBUF DMA partition-broadcast)
  out_partial[q, :] = sum_h o_h.T @ Wo_h  (K=32 row-tiled accumulation)
"""

import numpy as np
import ml_dtypes

import concourse.bass as bass
import concourse.tile as tile
from concourse import bacc, mybir
from concourse.bass_utils import run_bass_kernel_spmd
from concourse._compat import with_exitstack
from contextlib import ExitStack

B, D = 4, 256
H = 8
PROJ = 256
DH = PROJ // H            # 32
NCORES = 8
HPC = H // 2              # heads per core
PC = HPC * DH             # projection cols per core = 128
QB = 512                  # q block (PE moving dim / PSUM bank)
KBK = 128                 # k block
MASKBIAS = -60000.0       # exact in fp16; exp() underflows to exactly 0

# Fraction of (j, kb, pair) tiles whose mask-bias is applied on the DVE
# instead of injected on the PE: tiles with (idx % MASK_MOD) < MASK_DVE.
MASK_DVE = 2
MASK_MOD = 8

# Col-tiled AV + DMA-broadcast normalize (new structure) vs per-head AV
# banks at base 0 + ones-matmul broadcast (V1 structure, known-good on HW).
COLTILE = False

F32 = mybir.dt.float32
F16 = mybir.dt.float16
Identity = mybir.ActivationFunctionType.Identity
Exp = mybir.ActivationFunctionType.Exp
ts = bass.ts


@with_exitstack
def _emit(ctx: ExitStack, tc: tile.TileContext, t: dict, S: int):
    nc = tc.nc
    NQB = S // QB
    NKB = S // KBK

    wt = ctx.enter_context(tc.tile_pool(name="wt", bufs=1))
    sb = ctx.enter_context(tc.tile_pool(name="sb", bufs=1))
    wexp = ctx.enter_context(tc.tile_pool(name="wexp", bufs=3))
    mbp = ctx.enter_context(tc.tile_pool(name="mbp", bufs=3))
    ps = ctx.enter_context(tc.tile_pool(name="ps", bufs=3, space="PSUM"))
    avps = ctx.enter_context(tc.tile_pool(name="avps", bufs=2, space="PSUM"))

    # ---- persistent activations ----
    qT = sb.tile([128, S], F16)          # [proj_col, q]
    kT = sb.tile([128, S], F16)          # [proj_col, k]
    vaug = sb.tile([128, HPC, NKB, 33], F16)  # [k_in_blk, head, k_blk, dh+1]
    oT4 = sb.tile([32, HPC, S], F16)     # per-head attn out, rows 0-31
    den4 = sb.tile([128, HPC, QB], F16)  # dens at partition 32

    # ---- constants ----
    wq_s = wt.tile([128, 2, PC], F16)
    wk_s = wt.tile([128, 2, PC], F16)
    wv_s = wt.tile([128, 2, PC], F16)
    for c in range(2):
        nc.sync.dma_start(out=wq_s[:, c, :], in_=t["wq"][ts(c, 128), :])
        nc.sync.dma_start(out=wk_s[:, c, :], in_=t["wk"][ts(c, 128), :])
        nc.sync.dma_start(out=wv_s[:, c, :], in_=t["wv"][ts(c, 128), :])
    bq_s = wt.tile([128, 1], F32)
    bk_s = wt.tile([128, 1], F32)
    nc.sync.dma_start(out=bq_s[:], in_=t["bq"][:, :])
    nc.sync.dma_start(out=bk_s[:], in_=t["bk"][:, :])
    bv_bc = wt.tile([128, PC], F32)
    nc.sync.dma_start(out=bv_bc[:], in_=t["bv"].to_broadcast([128, PC]))
    id_s = wt.tile([128, 128], F16)
    nc.sync.dma_start(out=id_s[:], in_=t["ident"][:, :])
    ones32 = wt.tile([128, 32], F16)
    nc.sync.dma_start(out=ones32[:], in_=t["ones32"][:, :])
    wo4_s = wt.tile([32, HPC, D], F16)
    nc.sync.dma_start(out=wo4_s[:], in_=t["wo4"][:, :, :])
    load_vones = wt.tile([128, HPC, NKB, 1], F16)
    nc.sync.dma_start(out=load_vones[:], in_=t["vones"][:, :, :, :])
    nc.vector.tensor_copy(out=vaug[:, :, :, 32:33], in_=load_vones[:])

    with tc.tile_pool(name="xin", bufs=1) as xin:
        xq_s = xin.tile([128, 2, S], F16)
        xk_s = xin.tile([128, 2, S], F16)
        xv_s = xin.tile([128, 2, S], F16)
        for c in range(2):
            nc.sync.dma_start(out=xq_s[:, c, :], in_=t["xq"][ts(c, 128), :])
            nc.sync.dma_start(out=xk_s[:, c, :], in_=t["xk"][ts(c, 128), :])
            nc.sync.dma_start(out=xv_s[:, c, :], in_=t["xv"][ts(c, 128), :])

        # ---- q/k projections: psum = W.T @ xT  -> [proj, S] ----
        for dst, xs, ws, bs in ((qT, xq_s, wq_s, bq_s), (kT, xk_s, wk_s, bk_s)):
            for j in range(NQB):
                p = ps.tile([128, 2, QB], F32, tag="mm")
                for c in range(2):
                    nc.tensor.matmul(
                        p[:, 0, :],
                        lhsT=ws[:, c, :],
                        rhs=xs[:, c, ts(j, QB)],
                        start=(c == 0),
                        stop=(c == 1),
                    )
                nc.scalar.activation(
                    out=dst[:, ts(j, QB)], in_=p[:, 0, :],
                    func=Identity, bias=bs[:, 0:1], scale=1.0,
                )

        # ---- v projection in natural layout ----
        for sbk in range(NKB):
            p = ps.tile([128, 2, QB], F32, tag="mm")
            for c in range(2):
                nc.tensor.matmul(
                    p[:, 0, 0:PC],
                    lhsT=xv_s[:, c, ts(sbk, 128)],
                    rhs=wv_s[:, c, :],
                    start=(c == 0),
                    stop=(c == 1),
                )
            nc.vector.tensor_add(
                vaug[:, :, sbk, 0:32],
                p[:, 0, 0:PC].rearrange("p (h d) -> p h d", h=HPC),
                bv_bc[:, :].rearrange("p (h d) -> p h d", h=HPC),
            )

    # ---- attention main loop: one pass per head-pair, kb inner ----
    for j in range(NQB):
        for pair in range(2):
            av = [avps.tile([128, QB], F32, tag="av", name=f"av{i}")
                  for i in range(2)]
            for kb in range(NKB):
                mbt = mbp.tile([128, QB], F16)
                nc.sync.dma_start(out=mbt[:],
                                  in_=t["mb"][ts(kb, 128), ts(j, QB)])
                on_dve = ((j * NKB + kb) + pair) % MASK_MOD < MASK_DVE
                sc = ps.tile([128, 2, QB], F32, tag="mm")
                for i in range(2):
                    h = pair * 2 + i
                    if not on_dve:
                        nc.tensor.matmul(
                            sc[:, i, :], lhsT=id_s[:], rhs=mbt[:],
                            start=True, stop=False,
                        )
                    nc.tensor.matmul(
                        sc[:, i, :],
                        lhsT=kT[32 * h:32 * h + 32, ts(kb, 128)],
                        rhs=qT[32 * h:32 * h + 32, ts(j, QB)],
                        start=on_dve, stop=True,
                        tile_position=(32 * h, 0),
                    )
                w = wexp.tile([128, 2, QB], F16, tag="w")
                if on_dve:
                    sm = wexp.tile([128, 2, QB], F32, tag="sm")
                    nc.vector.tensor_add(
                        sm[:],
                        sc[:],
                        mbt[:].rearrange("p (o n) -> p o n", o=1)
                              .to_broadcast([128, 2, QB]),
                    )
                    nc.scalar.activation(out=w[:], in_=sm[:], func=Exp)
                else:
                    nc.scalar.activation(out=w[:], in_=sc[:], func=Exp)
                for i in range(2):
                    h = pair * 2 + i
                    nc.tensor.matmul(
                        av[i][0:33, :],
                        lhsT=vaug[:, h, kb, :],
                        rhs=w[:, i, :],
                        start=(kb == 0),
                        stop=(kb == NKB - 1),
                    )
            # ---- normalize this pair: oT4 rows = av rows * (1/den) ----
            for i in range(2):
                h = pair * 2 + i
                nc.vector.tensor_copy(
                    out=den4[32:33, h, :],
                    in_=av[i][32:33, :],
                )
            pb = ps.tile([128, 2, QB], F32, tag="mm")
            for i in range(2):
                h = pair * 2 + i
                nc.tensor.matmul(
                    pb[0:32, i, :],
                    lhsT=ones32[32:33, :],
                    rhs=den4[32:33, h, :],
                    start=True, stop=True,
                )
            rec = wexp.tile([32, 2, QB], F32, tag="rec")
            nc.vector.reciprocal_approx_fast(rec[:], pb[0:32, :, :])
            for i in range(2):
                h = pair * 2 + i
                nc.vector.tensor_mul(
                    oT4[0:32, h, ts(j, QB)],
                    av[i][0:32, :],
                    rec[:, i, :],
                )

    # ---- output projection: out[q, :] = sum_h oT_h.T @ wo_h ----
    for qb in range(S // 128):
        p = ps.tile([128, 2, QB], F32, tag="mm")
        for h in range(HPC):
            nc.tensor.matmul(
                p[:, 0, 0:D],
                lhsT=oT4[0:32, h, ts(qb, 128)],
                rhs=wo4_s[:, h, :],
                start=(h == 0), stop=(h == HPC - 1),
            )
        ob = wexp.tile([128, D], F32, tag="outbuf")
        nc.vector.tensor_copy(out=ob[:], in_=p[:, 0, 0:D])
        nc.sync.dma_start(out=t["out"][ts(qb, 128), :], in_=ob[:])


def build(S: int = 2048):
    nc = bacc.Bacc("TRN2", target_bir_lowering=False, debug=False,
                   num_devices=NCORES)
    t = {}
    t["xq"] = nc.dram_tensor("xq", [D, S], F16, kind="ExternalInput").ap()
    t["xk"] = nc.dram_tensor("xk", [D, S], F16, kind="ExternalInput").ap()
    t["xv"] = nc.dram_tensor("xv", [D, S], F16, kind="ExternalInput").ap()
    t["wq"] = nc.dram_tensor("wq", [D, PC], F16, kind="ExternalInput").ap()
    t["wk"] = nc.dram_tensor("wk", [D, PC], F16, kind="ExternalInput").ap()
    t["wv"] = nc.dram_tensor("wv", [D, PC], F16, kind="ExternalInput").ap()
    t["wo4"] = nc.dram_tensor("wo4", [32, HPC, D], F16,
                              kind="ExternalInput").ap()
    t["ones32"] = nc.dram_tensor("ones32", [128, 32], F16,
                                 kind="ExternalInput").ap()
    t["bq"] = nc.dram_tensor("bq", [PC, 1], F32, kind="ExternalInput").ap()
    t["bk"] = nc.dram_tensor("bk", [PC, 1], F32, kind="ExternalInput").ap()
    t["bv"] = nc.dram_tensor("bv", [1, PC], F32, kind="ExternalInput").ap()
    t["ident"] = nc.dram_tensor("ident", [128, 128], F16,
                                kind="ExternalInput").ap()
    t["mb"] = nc.dram_tensor("mb", [S, S], F16, kind="ExternalInput").ap()
    t["vones"] = nc.dram_tensor("vones", [128, HPC, S // 128, 1], F16,
                                kind="ExternalInput").ap()
    t["out"] = nc.dram_tensor("out", [S, D], F32, kind="ExternalOutput").ap()

    with tile.TileContext(nc) as tc:
        _emit(tc, t, S)
    nc.compile()
    return nc


_NC_CACHE = {}


def _get_nc(S):
    if S not in _NC_CACHE:
        _NC_CACHE[S] = build(S)
    return _NC_CACHE[S]


def _pack_wo4(wo_slice):
    """[PC, D] -> [32, HPC, D] per-head rows."""
    w = np.zeros((32, HPC, D), np.float32)
    for h in range(HPC):
        w[:, h, :] = wo_slice[32 * h:32 * h + 32, :]
    return w


def make_in_maps(queries, keys, values, mask, Wq, bq, Wk, bk, Wv, bv, Wo, bo):
    queries = np.asarray(queries, np.float32)
    keys = np.asarray(keys, np.float32)
    values = np.asarray(values, np.float32)
    mask = np.asarray(mask)
    Wq, Wk, Wv, Wo = (np.asarray(a, np.float32) for a in (Wq, Wk, Wv, Wo))
    bq, bk, bv, bo = (np.asarray(a, np.float32) for a in (bq, bk, bv, bo))
    S = queries.shape[1]
    sc = np.float32(1.0) / np.sqrt(np.float32(PROJ))
    f16 = np.float16
    ident = np.eye(128, dtype=f16)
    in_maps = []
    for c in range(NCORES):
        b = c // 2
        p0 = PC * (c % 2)
        mbt = np.where(mask[b, 0].T, np.float32(0), np.float32(MASKBIAS))
        im = {
            "xq": np.ascontiguousarray(queries[b].T).astype(f16),
            "xk": np.ascontiguousarray(keys[b].T).astype(f16),
            "xv": np.ascontiguousarray(values[b].T).astype(f16),
            "wq": (Wq[:, p0:p0 + PC] * sc).astype(f16),
            "wk": Wk[:, p0:p0 + PC].astype(f16),
            "wv": Wv[:, p0:p0 + PC].astype(f16),
            "bq": np.ascontiguousarray((bq[p0:p0 + PC] * sc).reshape(PC, 1)),
            "bk": np.ascontiguousarray(bk[p0:p0 + PC].reshape(PC, 1)),
            "bv": np.ascontiguousarray(bv[p0:p0 + PC].reshape(1, PC)),
            "ident": ident,
            "mb": mbt.astype(f16),
            "vones": np.ones((128, HPC, S // 128, 1), f16),
            "wo4": _pack_wo4(Wo[p0:p0 + PC, :]).astype(f16),
            "ones32": np.ones((128, 32), f16),
        }
        in_maps.append(im)
    return in_maps


def run(inputs, trace=False):
    S = np.asarray(inputs["queries"]).shape[1]
    nc = _get_nc(S)
    in_maps = make_in_maps(**inputs)
    res = run_bass_kernel_spmd(nc, in_maps, core_ids=list(range(NCORES)),
                               trace=trace)
    parts = [np.asarray(r["out"], np.float32) for r in res.results]
    bo = np.asarray(inputs["bo"], np.float32)
    out = np.zeros((B, S, D), np.float32)
    for b in range(B):
        out[b] = parts[2 * b] + parts[2 * b + 1] + bo[None, :]
    return out, res


def kernel(**inputs) -> np.ndarray:
    out, _ = run(inputs, trace=False)
    return out



# revision 2
# speedup vs baseline: 1.3578x; 1.3578x over previous
"""Multi-head attention kernel for Trainium2 (8 NeuronCores, SPMD).

Sharding: core c handles batch b=c//2 and 4 of the 8 heads
(projection columns 128*(c%2) .. +128).  Each core computes a partial
output projection (contracting only its own 128 head-dims); the host sums
the two partials per batch and adds bo.

Device algorithm (per core), S=2048, D=256, 4 heads of dh=32.  All hot
matmul operands are fp16 (1 cyc/row on the PE, PSUM accumulation fp32):
  qT/kT = (x @ W).T computed directly in [proj, S] layout
  v     = x @ Wv in natural [S, proj] layout, + ones column (den fused)
  scores^T[k,q] = sum_d kT[d,k] qT[d,q]   (4 heads row-tiled on the PE)
  mask applied per (j,kb) tile on one of two paths (split tunable):
    - PE: fp16 identity-matmul injects bias 0/-60000 into the scores PSUM
      before exp (exp underflows to exactly 0) — keeps the PE busy so it
      holds its high p-state clock, or
    - DVE: w = exp(s) * m01 with m01 in {0,1} fp16 (2x DVE mode)
  w = exp(scores) on ScalarE, fp16 out
  avT[d,q] + den[q] = [v | ones].T @ w    (fp32 PSUM accum over kb)
  o = av * (1/den)  (reciprocal_approx_fast, DVE)
  out_partial[q, :] = sum_h o_h.T @ Wo_h  (K=32 row-tiled accumulation,
  interleaved per j-block to overlap with the next block's softmax)
"""

import numpy as np

import concourse.bass as bass
import concourse.tile as tile
from concourse import bacc, mybir
from concourse.bass_utils import run_bass_kernel_spmd
from concourse._compat import with_exitstack
from contextlib import ExitStack

B, D = 4, 256
H = 8
PROJ = 256
DH = PROJ // H            # 32
NCORES = 8
HPC = H // 2              # heads per core
PC = HPC * DH             # projection cols per core = 128
QB = 512                  # q block (PE moving dim / PSUM bank)
KBK = 128                 # k block
MASKBIAS = -60000.0       # exact in fp16; exp() underflows to exactly 0

# Fraction of (j, kb) mask tiles injected on the PE (identity matmul into
# the scores PSUM) instead of multiplied on the DVE post-exp: tiles with
# (j*NKB + kb) % MASK_MOD < MASK_PE go to the PE.  The PE path costs PE
# rows but keeps the tensor engine saturated (p-state); the DVE path costs
# vector cycles.  Both pairs of heads share the tile, so the choice is per
# (j, kb).
MASK_PE = 1
MASK_MOD = 4

F32 = mybir.dt.float32
F16 = mybir.dt.float16
Identity = mybir.ActivationFunctionType.Identity
Exp = mybir.ActivationFunctionType.Exp
ts = bass.ts


def _mask_on_pe(j, kb, nkb):
    return (j * nkb + kb) % MASK_MOD < MASK_PE


@with_exitstack
def _emit(ctx: ExitStack, tc: tile.TileContext, t: dict, S: int):
    nc = tc.nc
    NQB = S // QB
    NKB = S // KBK

    wt = ctx.enter_context(tc.tile_pool(name="wt", bufs=1))
    sb = ctx.enter_context(tc.tile_pool(name="sb", bufs=1))
    wexp = ctx.enter_context(tc.tile_pool(name="wexp", bufs=3))
    wmul = ctx.enter_context(tc.tile_pool(name="wmul", bufs=3))
    mkp = ctx.enter_context(tc.tile_pool(name="mkp", bufs=2))
    ps = ctx.enter_context(tc.tile_pool(name="ps", bufs=3, space="PSUM"))
    avps = ctx.enter_context(tc.tile_pool(name="avps", bufs=2, space="PSUM"))

    # ---- persistent activations ----
    qT = sb.tile([128, S], F16)          # [proj_col, q]
    kT = sb.tile([128, S], F16)          # [proj_col, k]
    vaug = sb.tile([128, HPC, NKB, 33], F16)  # [k_in_blk, head, k_blk, dh+1]
    oT4 = sb.tile([32, HPC, S], F16)     # per-head attn out, rows 0-31
    den4 = sb.tile([128, HPC, QB], F16)  # dens at partition 32

    # ---- constants ----
    wq_s = wt.tile([128, 2, PC], F16)
    wk_s = wt.tile([128, 2, PC], F16)
    wv_s = wt.tile([128, 2, PC], F16)
    for c in range(2):
        nc.sync.dma_start(out=wq_s[:, c, :], in_=t["wq"][ts(c, 128), :])
        nc.sync.dma_start(out=wk_s[:, c, :], in_=t["wk"][ts(c, 128), :])
        nc.sync.dma_start(out=wv_s[:, c, :], in_=t["wv"][ts(c, 128), :])
    bq_s = wt.tile([128, 1], F32)
    bk_s = wt.tile([128, 1], F32)
    nc.sync.dma_start(out=bq_s[:], in_=t["bq"][:, :])
    nc.sync.dma_start(out=bk_s[:], in_=t["bk"][:, :])
    bv_bc = wt.tile([128, PC], F32)
    nc.sync.dma_start(out=bv_bc[:], in_=t["bv"].to_broadcast([128, PC]))
    id_s = wt.tile([128, 128], F16)
    nc.sync.dma_start(out=id_s[:], in_=t["ident"][:, :])
    ones32 = wt.tile([128, 32], F16)
    nc.sync.dma_start(out=ones32[:], in_=t["ones32"][:, :])
    wo4_s = wt.tile([32, HPC, D], F16)
    nc.sync.dma_start(out=wo4_s[:], in_=t["wo4"][:, :, :])
    load_vones = wt.tile([128, HPC, NKB, 1], F16)
    nc.sync.dma_start(out=load_vones[:], in_=t["vones"][:, :, :, :])
    nc.vector.tensor_copy(out=vaug[:, :, :, 32:33], in_=load_vones[:])

    def load_masks(j):
        """Per-j mask tiles; encoding depends on the tile's mask path."""
        mk = mkp.tile([128, NKB, QB], F16, tag="mk")
        for kb in range(NKB):
            src = t["mb"] if _mask_on_pe(j, kb, NKB) else t["m01"]
            nc.sync.dma_start(out=mk[:, kb, :],
                              in_=src[ts(kb, 128), ts(j, QB)])
        return mk

    mk_cur = load_masks(0)

    with tc.tile_pool(name="xin", bufs=1) as xin:
        xq_s = xin.tile([128, 2, S], F16)
        xk_s = xin.tile([128, 2, S], F16)
        xv_s = xin.tile([128, 2, S], F16)
        for c in range(2):
            nc.sync.dma_start(out=xq_s[:, c, :], in_=t["xq"][ts(c, 128), :])
            nc.sync.dma_start(out=xk_s[:, c, :], in_=t["xk"][ts(c, 128), :])
            nc.sync.dma_start(out=xv_s[:, c, :], in_=t["xv"][ts(c, 128), :])

        # ---- q/k projections: psum = W.T @ xT  -> [proj, S] ----
        for dst, xs, ws, bs in ((qT, xq_s, wq_s, bq_s), (kT, xk_s, wk_s, bk_s)):
            for j in range(NQB):
                p = ps.tile([128, 2, QB], F32, tag="mm")
                for c in range(2):
                    nc.tensor.matmul(
                        p[:, 0, :],
                        lhsT=ws[:, c, :],
                        rhs=xs[:, c, ts(j, QB)],
                        start=(c == 0),
                        stop=(c == 1),
                    )
                nc.scalar.activation(
                    out=dst[:, ts(j, QB)], in_=p[:, 0, :],
                    func=Identity, bias=bs[:, 0:1], scale=1.0,
                )

        # ---- v projection in natural layout ----
        for sbk in range(NKB):
            p = ps.tile([128, 2, QB], F32, tag="mm")
            for c in range(2):
                nc.tensor.matmul(
                    p[:, 0, 0:PC],
                    lhsT=xv_s[:, c, ts(sbk, 128)],
                    rhs=wv_s[:, c, :],
                    start=(c == 0),
                    stop=(c == 1),
                )
            nc.vector.tensor_add(
                vaug[:, :, sbk, 0:32],
                p[:, 0, 0:PC].rearrange("p (h d) -> p h d", h=HPC),
                bv_bc[:, :].rearrange("p (h d) -> p h d", h=HPC),
            )

    # ---- attention main loop: one pass per head-pair, kb inner ----
    for j in range(NQB):
        if j + 1 < NQB:
            mk_next = load_masks(j + 1)
        for pair in range(2):
            av = [avps.tile([128, QB], F32, tag="av", name=f"av{i}")
                  for i in range(2)]
            for kb in range(NKB):
                on_pe = _mask_on_pe(j, kb, NKB)
                sc = ps.tile([128, 2, QB], F32, tag="mm")
                for i in range(2):
                    h = pair * 2 + i
                    if on_pe:
                        nc.tensor.matmul(
                            sc[:, i, :], lhsT=id_s[:],
                            rhs=mk_cur[:, kb, :],
                            start=True, stop=False,
                        )
                    nc.tensor.matmul(
                        sc[:, i, :],
                        lhsT=kT[32 * h:32 * h + 32, ts(kb, 128)],
                        rhs=qT[32 * h:32 * h + 32, ts(j, QB)],
                        start=not on_pe, stop=True,
                        tile_position=(32 * h, 0),
                    )
                we = wexp.tile([128, 2, QB], F16, tag="we")
                nc.scalar.activation(out=we[:], in_=sc[:], func=Exp)
                if on_pe:
                    w = we
                else:
                    w = wmul.tile([128, 2, QB], F16, tag="w")
                    nc.vector.tensor_mul(
                        w[:],
                        we[:],
                        mk_cur[:, kb, :].rearrange("p (o n) -> p o n", o=1)
                                        .to_broadcast([128, 2, QB]),
                    )
                for i in range(2):
                    h = pair * 2 + i
                    nc.tensor.matmul(
                        av[i][0:33, :],
                        lhsT=vaug[:, h, kb, :],
                        rhs=w[:, i, :],
                        start=(kb == 0),
                        stop=(kb == NKB - 1),
                    )
            # ---- normalize this pair: oT4 rows = av rows * (1/den) ----
            for i in range(2):
                h = pair * 2 + i
                nc.vector.tensor_copy(
                    out=den4[32:33, h, :],
                    in_=av[i][32:33, :],
                )
            pb = ps.tile([128, 2, QB], F32, tag="mm")
            for i in range(2):
                h = pair * 2 + i
                nc.tensor.matmul(
                    pb[0:32, i, :],
                    lhsT=ones32[32:33, :],
                    rhs=den4[32:33, h, :],
                    start=True, stop=True,
                )
            rec = wexp.tile([32, 2, QB], F32, tag="rec")
            nc.vector.reciprocal_approx_fast(rec[:], pb[0:32, :, :])
            for i in range(2):
                h = pair * 2 + i
                nc.vector.tensor_mul(
                    oT4[0:32, h, ts(j, QB)],
                    av[i][0:32, :],
                    rec[:, i, :],
                )
        # ---- output projection for this j-block (fills PE bubbles) ----
        for qq in range(QB // 128):
            qb = j * (QB // 128) + qq
            p = ps.tile([128, 2, QB], F32, tag="mm")
            for h in range(HPC):
                nc.tensor.matmul(
                    p[:, 0, 0:D],
                    lhsT=oT4[0:32, h, ts(qb, 128)],
                    rhs=wo4_s[:, h, :],
                    start=(h == 0), stop=(h == HPC - 1),
                )
            ob = wexp.tile([128, D], F32, tag="outbuf")
            nc.vector.tensor_copy(out=ob[:], in_=p[:, 0, 0:D])
            nc.sync.dma_start(out=t["out"][ts(qb, 128), :], in_=ob[:])
        if j + 1 < NQB:
            mk_cur = mk_next


def build(S: int = 2048):
    nc = bacc.Bacc("TRN2", target_bir_lowering=False, debug=False,
                   num_devices=NCORES)
    t = {}
    t["xq"] = nc.dram_tensor("xq", [D, S], F16, kind="ExternalInput").ap()
    t["xk"] = nc.dram_tensor("xk", [D, S], F16, kind="ExternalInput").ap()
    t["xv"] = nc.dram_tensor("xv", [D, S], F16, kind="ExternalInput").ap()
    t["wq"] = nc.dram_tensor("wq", [D, PC], F16, kind="ExternalInput").ap()
    t["wk"] = nc.dram_tensor("wk", [D, PC], F16, kind="ExternalInput").ap()
    t["wv"] = nc.dram_tensor("wv", [D, PC], F16, kind="ExternalInput").ap()
    t["wo4"] = nc.dram_tensor("wo4", [32, HPC, D], F16,
                              kind="ExternalInput").ap()
    t["ones32"] = nc.dram_tensor("ones32", [128, 32], F16,
                                 kind="ExternalInput").ap()
    t["bq"] = nc.dram_tensor("bq", [PC, 1], F32, kind="ExternalInput").ap()
    t["bk"] = nc.dram_tensor("bk", [PC, 1], F32, kind="ExternalInput").ap()
    t["bv"] = nc.dram_tensor("bv", [1, PC], F32, kind="ExternalInput").ap()
    t["ident"] = nc.dram_tensor("ident", [128, 128], F16,
                                kind="ExternalInput").ap()
    t["mb"] = nc.dram_tensor("mb", [S, S], F16, kind="ExternalInput").ap()
    t["m01"] = nc.dram_tensor("m01", [S, S], F16, kind="ExternalInput").ap()
    t["vones"] = nc.dram_tensor("vones", [128, HPC, S // 128, 1], F16,
                                kind="ExternalInput").ap()
    t["out"] = nc.dram_tensor("out", [S, D], F32, kind="ExternalOutput").ap()

    with tile.TileContext(nc) as tc:
        _emit(tc, t, S)
    nc.compile()
    return nc


_NC_CACHE = {}


def _get_nc(S):
    if S not in _NC_CACHE:
        _NC_CACHE[S] = build(S)
    return _NC_CACHE[S]


def _pack_wo4(wo_slice):
    """[PC, D] -> [32, HPC, D] per-head rows."""
    w = np.zeros((32, HPC, D), np.float32)
    for h in range(HPC):
        w[:, h, :] = wo_slice[32 * h:32 * h + 32, :]
    return w


def make_in_maps(queries, keys, values, mask, Wq, bq, Wk, bk, Wv, bv, Wo, bo):
    queries = np.asarray(queries, np.float32)
    keys = np.asarray(keys, np.float32)
    values = np.asarray(values, np.float32)
    mask = np.asarray(mask)
    Wq, Wk, Wv, Wo = (np.asarray(a, np.float32) for a in (Wq, Wk, Wv, Wo))
    bq, bk, bv, bo = (np.asarray(a, np.float32) for a in (bq, bk, bv, bo))
    S = queries.shape[1]
    sc = np.float32(1.0) / np.sqrt(np.float32(PROJ))
    f16 = np.float16
    ident = np.eye(128, dtype=f16)
    in_maps = []
    for c in range(NCORES):
        b = c // 2
        p0 = PC * (c % 2)
        maskT = mask[b, 0].T
        mbt = np.where(maskT, np.float16(0), np.float16(MASKBIAS))
        m01 = maskT.astype(f16)
        im = {
            "xq": np.ascontiguousarray(queries[b].T).astype(f16),
            "xk": np.ascontiguousarray(keys[b].T).astype(f16),
            "xv": np.ascontiguousarray(values[b].T).astype(f16),
            "wq": (Wq[:, p0:p0 + PC] * sc).astype(f16),
            "wk": Wk[:, p0:p0 + PC].astype(f16),
            "wv": Wv[:, p0:p0 + PC].astype(f16),
            "bq": np.ascontiguousarray((bq[p0:p0 + PC] * sc).reshape(PC, 1)),
            "bk": np.ascontiguousarray(bk[p0:p0 + PC].reshape(PC, 1)),
            "bv": np.ascontiguousarray(bv[p0:p0 + PC].reshape(1, PC)),
            "ident": ident,
            "mb": mbt.astype(f16),
            "m01": m01,
            "vones": np.ones((128, HPC, S // 128, 1), f16),
            "wo4": _pack_wo4(Wo[p0:p0 + PC, :]).astype(f16),
            "ones32": np.ones((128, 32), f16),
        }
        in_maps.append(im)
    return in_maps


def run(inputs, trace=False):
    S = np.asarray(inputs["queries"]).shape[1]
    nc = _get_nc(S)
    in_maps = make_in_maps(**inputs)
    res = run_bass_kernel_spmd(nc, in_maps, core_ids=list(range(NCORES)),
                               trace=trace)
    parts = [np.asarray(r["out"], np.float32) for r in res.results]
    bo = np.asarray(inputs["bo"], np.float32)
    out = np.zeros((B, S, D), np.float32)
    for b in range(B):
        out[b] = parts[2 * b] + parts[2 * b + 1] + bo[None, :]
    return out, res


def kernel(**inputs) -> np.ndarray:
    out, _ = run(inputs, trace=False)
    return out
